# revision 2
# baseline (speedup 1.0000x reference)
"""MoE FFN (top-2 routing, 8 experts) on 8 Trainium2 NeuronCores.

Strategy (expert parallelism, per the sharding hint):
  - Host computes router logits / top-2 / softmax (tiny: T x E) and
    dispatches tokens: expert e's tokens are gathered into a padded
    [H, C] batch for core e (C = common capacity).
  - Core e runs the dense FFN for its expert on its gathered tokens:
        yT = ( GELU_tanh(x @ W1 + b1) @ W2 + b2 )^T
    computed fully transposed ([F,c] then [H,c]) so both matmuls use
    the weights as the stationary operand and no on-device transposes
    are needed. Matmul operands are fp16 (same PE rate as bf16 but 8x
    finer mantissa); accumulation is fp32 in PSUM.
  - The per-token combine weight is applied on the HOST during the
    scatter-add (cheap, and skips a whole [H, C] vector pass + the
    wtb upload on device).

Performance structure (v2, vs the 165 us baseline):
  - C is padded to a multiple of 4 (not 128) and split into equal
    chunks <= 512 wide, so the PE streams ~288*C rows instead of
    288*pad128(C): for this input C drops 1152 -> 1068 (~9.7 us).
  - DMA is issued in consumption order with a small first W1 piece and
    the first x chunk heading the two HWDGE rings, so the first real
    matmul starts at ~11 us instead of ~16.6 us.
  - Phase B of chunk 0 iterates fk-outer across 6 PSUM banks so W2
    streaming is spread over the whole phase (no burst demand); later
    chunks go hn-outer so the epilogue+store of each output row-tile
    overlaps the remaining matmuls, keeping the kernel tail short.
  - PE warmup (~3.5 us of dummy matmuls) ramps the clock 1.2->2.4 GHz
    while the first DMAs are in flight; the clock stays up afterwards.

Self-contained: hardcodes the problem shapes (H=768, F=3072, E=8, K=2).
"""

import os
import time

import numpy as np

H = 768
F = 3072
E = 8
K = 2
N_CORES = 8
P = 128
FM = F // P   # 24 f row-tiles
HK = H // P   # 6 contraction tiles for x@W1
HN = H // P   # 6 output row-tiles of yT

PRECISION = os.environ.get("MOE_PRECISION", "fp16")  # "fp16" | "bf16" | "fp32"
WARMUP_MM = int(os.environ.get("MOE_WARMUP_MM", "10"))


def _chunks(C):
    """Split C columns into near-equal chunks of width <= 512 (PSUM bank)."""
    n = max(1, -(-C // 512))
    base, rem = divmod(C, n)
    ws = [base + 1] * rem + [base] * (n - rem)
    out, c0 = [], 0
    for w in ws:
        out.append((c0, w))
        c0 += w
    return out


# ---------------------------------------------------------------------------
# Bass/Tile device kernel
# ---------------------------------------------------------------------------

def _build_bass(C, precision=None):
    from contextlib import ExitStack

    import concourse.bass as bass  # noqa: F401
    import concourse.tile as tile
    from concourse import bacc, mybir
    from concourse._compat import with_exitstack

    precision = precision or PRECISION
    f32 = mybir.dt.float32
    mdt = {"bf16": mybir.dt.bfloat16, "fp16": mybir.dt.float16,
           "fp32": f32}[precision]

    chunks = _chunks(C)
    WMAX = max(w for _, w in chunks)
    W0 = chunks[0][1]

    nc = bacc.Bacc("TRN2", target_bir_lowering=False, debug=False,
                   num_devices=N_CORES)
    xgT = nc.dram_tensor("xgt", [H, C], mdt, kind="ExternalInput").ap()
    w1 = nc.dram_tensor("w1", [H, F], mdt, kind="ExternalInput").ap()
    w2 = nc.dram_tensor("w2", [F, H], mdt, kind="ExternalInput").ap()
    # packed fp32 constants: [b1t | b2t] along the free dim
    cpk = nc.dram_tensor("cpk", [P, FM + HN], f32,
                         kind="ExternalInput").ap()
    y = nc.dram_tensor("y", [H, C], mdt, kind="ExternalOutput").ap()

    gelu = mybir.ActivationFunctionType.Gelu_apprx_tanh
    ident = mybir.ActivationFunctionType.Identity

    # W1 column pieces (fm-consumption order): a small head piece so the
    # first matmul can start early, then 256-col pieces.
    assert F == 3072
    w1_pieces = [(0, P)] + [(P + 256 * i, 256) for i in range(11)] \
        + [(F - P, P)]

    def w1_piece_of(fm):
        if fm == 0:
            return 0, 0
        i = (fm + 1) // 2
        off = 0 if fm % 2 == 1 else P
        return i, off

    # W2 row-groups: contiguous [512, H] blocks (4 fk tiles each).
    W2G = 4
    n_w2p = FM // W2G  # 6

    @with_exitstack
    def body(ctx: ExitStack, tc: tile.TileContext):
        const = ctx.enter_context(tc.tile_pool(name="const", bufs=1))
        w1pool = ctx.enter_context(tc.tile_pool(name="w1pool", bufs=1))
        w2pool = ctx.enter_context(tc.tile_pool(name="w2pool", bufs=1))
        xp = ctx.enter_context(tc.tile_pool(name="xp", bufs=1))
        hp = ctx.enter_context(tc.tile_pool(name="hp", bufs=1))
        yp = ctx.enter_context(tc.tile_pool(name="yp", bufs=3))
        psAp = ctx.enter_context(tc.tile_pool(name="psA", bufs=2, space="PSUM"))
        psBp = ctx.enter_context(tc.tile_pool(name="psB", bufs=1, space="PSUM"))

        # --- PE warmup: ramp the HAM clock gate 1.2 -> 2.4 GHz during the
        # DMA-bound startup. The clock stays up once ramped.
        wtile = xp.tile([P, 512], mdt, tag="warm", name="warm")
        nc.vector.memset(wtile[:], 0.0)
        wps = psBp.tile([P, WMAX], f32, tag="psB0", name="warmps")
        for i in range(WARMUP_MM):
            nc.tensor.matmul(wps[:, :WMAX], lhsT=wtile[:, 0:P],
                             rhs=wtile[:, 0:WMAX],
                             start=(i == 0), stop=(i == WARMUP_MM - 1))

        # --- DMA staging. Two HWDGE rings (sync + scalar) share the HBM
        # pipe; jobs are enqueued in consumption order per ring.
        xg0 = xp.tile([P, HK, W0], mdt, tag="xg0", name="xg0")
        xgf = None
        if C > W0:
            xgf = xp.tile([P, HK, C - W0], mdt, tag="xgf", name="xgf")
        b12 = const.tile([P, FM + HN], f32, name="b12")
        b1s = b12[:, 0:FM]
        b2s = b12[:, FM:]
        w1t = [w1pool.tile([P, HK, wdt], mdt, tag=f"w1p{i}", name=f"w1p{i}")
               for i, (_, wdt) in enumerate(w1_pieces)]
        w2t = [w2pool.tile([P, W2G, H], mdt, tag=f"w2p{g}", name=f"w2p{g}")
               for g in range(n_w2p)]

        def w1_dma(ring, i):
            c0, wdt = w1_pieces[i]
            ring.dma_start(
                w1t[i][:], w1[:, c0:c0 + wdt].rearrange("(k p) f -> p k f", p=P))

        def w2_dma(ring, g):
            ring.dma_start(
                w2t[g][:],
                w2[g * W2G * P:(g + 1) * W2G * P, :].rearrange(
                    "(k p) f -> p k f", p=P))

        # scalar ring: b1/b2 first (needed by first activation), then the
        # small W1 head piece (gates the first matmul), odd W1 pieces,
        # even W2 groups.
        nc.scalar.dma_start(b12[:], cpk[:])
        w1_dma(nc.scalar, 0)
        # sync ring: first x chunk (gates the first matmul), even W1
        # pieces, the rest of x, odd W2 groups.
        nc.sync.dma_start(
            xg0[:], xgT[:, 0:W0].rearrange("(k p) c -> p k c", p=P))
        for i in range(1, len(w1_pieces)):
            w1_dma(nc.scalar if i % 2 == 1 else nc.sync, i)
        if xgf is not None:
            nc.sync.dma_start(
                xgf[:], xgT[:, W0:].rearrange("(k p) c -> p k c", p=P))
        for g in range(n_w2p):
            w2_dma(nc.scalar if g % 2 == 0 else nc.sync, g)

        def w1_tile(hk, fm):
            i, off = w1_piece_of(fm)
            return w1t[i][:, hk, off:off + P]

        def w2_tile(fk, hn):
            return w2t[fk // W2G][:, fk % W2G, hn * P:(hn + 1) * P]

        def xg_slice(hk, c0, w):
            if c0 < W0:
                return xg0[:, hk, c0:c0 + w]
            return xgf[:, hk, c0 - W0:c0 - W0 + w]

        yT = y.rearrange("(k p) c -> p k c", p=P)

        for ci, (c0, w) in enumerate(chunks):
            last = ci == len(chunks) - 1
            # ---- phase A: hT[f, c] = gelu((x@W1)[c, f] + b1[f]) ----
            hts = hp.tile([P, FM, WMAX], mdt, tag="hts", name="hts")
            for fm in range(FM):
                ps = psAp.tile([P, WMAX], f32, tag="psA", name="psA")
                for hk in range(HK):
                    nc.tensor.matmul(
                        ps[:, :w],
                        lhsT=w1_tile(hk, fm),
                        rhs=xg_slice(hk, c0, w),
                        start=(hk == 0), stop=(hk == HK - 1),
                    )
                nc.scalar.activation(hts[:, fm, :w], ps[:, :w], gelu,
                                     bias=b1s[:, fm:fm + 1])

            # ---- phase B: yT[h, c] = sum_f W2[f, h] * hT[f, c] (+b2) ----
            yo = yp.tile([P, HN, WMAX], mdt, tag="yout", name="yout")
            if ci == 0 and not last:
                # fk-outer across 6 PSUM banks: W2[fk] is consumed
                # progressively, so its DMA can stream during the phase.
                psBs = [psBp.tile([P, WMAX], f32, tag=f"psB{j}",
                                  name=f"psB{j}") for j in range(HN)]
                for fk in range(FM):
                    for hn in range(HN):
                        nc.tensor.matmul(
                            psBs[hn][:, :w],
                            lhsT=w2_tile(fk, hn),
                            rhs=hts[:, fk, :w],
                            start=(fk == 0), stop=(fk == FM - 1),
                        )
                for hn in range(HN):
                    nc.scalar.activation(yo[:, hn, :w], psBs[hn][:, :w],
                                         ident, bias=b2s[:, hn:hn + 1])
                nc.sync.dma_start(yT[:, :, c0:c0 + w], yo[:, :, :w])
            else:
                # hn-outer: epilogue + store of each row-tile overlap the
                # remaining matmuls (short kernel tail on the last chunk).
                for hn in range(HN):
                    ps = psBp.tile([P, WMAX], f32, tag=f"psB{hn % HN}",
                                   name=f"psBr{hn}")
                    for fk in range(FM):
                        nc.tensor.matmul(
                            ps[:, :w],
                            lhsT=w2_tile(fk, hn),
                            rhs=hts[:, fk, :w],
                            start=(fk == 0), stop=(fk == FM - 1),
                        )
                    nc.scalar.activation(yo[:, hn, :w], ps[:, :w],
                                         ident, bias=b2s[:, hn:hn + 1])
                    if last:
                        nc.sync.dma_start(
                            y[hn * P:(hn + 1) * P, c0:c0 + w],
                            yo[:, hn, :w])
                if not last:
                    nc.sync.dma_start(yT[:, :, c0:c0 + w], yo[:, :, :w])

    with tile.TileContext(nc) as tc:
        body(tc)
    nc.compile()
    return nc


# ---------------------------------------------------------------------------
# Host-side routing + dispatch
# ---------------------------------------------------------------------------

def _route(xf, gate_w):
    """Top-2 router in float64 for a numerically robust top-k set.

    Returns per-expert (token_idx, weight) lists.
    """
    logits = xf.astype(np.float64) @ gate_w.astype(np.float64)  # [T, E]
    top_idx = np.argpartition(logits, E - K, axis=1)[:, E - K:]  # [T, K]
    top_val = np.take_along_axis(logits, top_idx, axis=1)
    m = top_val.max(axis=1, keepdims=True)
    ex = np.exp(top_val - m)
    wts = ex / ex.sum(axis=1, keepdims=True)  # [T, K] float64

    toks, ws = [], []
    for e in range(E):
        mask = top_idx == e  # [T, K]
        rows = np.nonzero(mask.any(axis=1))[0]
        toks.append(rows)
        ws.append(wts[mask].astype(np.float32))
    return toks, ws


def _np_mdt():
    import ml_dtypes
    return {"bf16": ml_dtypes.bfloat16, "fp16": np.float16,
            "fp32": np.float32}[PRECISION]


def _make_in_maps(xf, W1, b1, W2, b2, toks, C):
    mdt = _np_mdt()
    b1a = np.asarray(b1, np.float32)
    b2a = np.asarray(b2, np.float32)
    in_maps = []
    for e in range(E):
        n_e = len(toks[e])
        xgT = np.zeros((H, C), mdt)
        xgT[:, :n_e] = xf[toks[e]].T.astype(mdt)
        cpk = np.concatenate([
            b1a[e].reshape(FM, P).T,
            b2a[e].reshape(HN, P).T,
        ], axis=1)
        in_maps.append({
            "xgt": xgT,
            "w1": np.asarray(W1[e], np.float32).astype(mdt),
            "w2": np.asarray(W2[e], np.float32).astype(mdt),
            "cpk": np.ascontiguousarray(cpk),
        })
    return in_maps


def _run(inputs, trace=False):
    global PRECISION
    from concourse.bass_utils import run_bass_kernel_spmd

    x, gate_w, W1, b1, W2, b2 = (inputs[k] for k in
                                 ("x", "gate_w", "W1", "b1", "W2", "b2"))
    x = np.asarray(x)
    Bb, S, Hd = x.shape
    assert Hd == H
    T = Bb * S
    xf = np.ascontiguousarray(x.reshape(T, Hd), dtype=np.float32)
    gate_w = np.asarray(gate_w, np.float32)

    # fp16 matmul operands need moderate dynamic range; fall back to
    # bf16 (full fp32 exponent range) if the data is far outside the
    # expected unit-scale regime.
    if PRECISION == "fp16":
        amax = max(float(np.abs(np.asarray(t)).max())
                   for t in (xf, W1, W2))
        if not np.isfinite(amax) or amax > 1e3:
            PRECISION = "bf16"

    toks, ws = _route(xf, gate_w)
    nmax = max(len(t) for t in toks)
    C = max(P, ((nmax + 3) // 4) * 4)
    in_maps = _make_in_maps(xf, W1, b1, W2, b2, toks, C)
    nc = _build_bass(C)

    kwargs = {}
    if trace:
        kwargs = dict(trace=True, trace_cores=list(range(N_CORES)))
    try:
        res = run_bass_kernel_spmd(nc, in_maps, core_ids=list(range(N_CORES)),
                                   **kwargs)
    except Exception:
        # One retry for transient device faults.
        time.sleep(5)
        res = run_bass_kernel_spmd(nc, in_maps, core_ids=list(range(N_CORES)),
                                   **kwargs)
    out = np.zeros((T, H), np.float32)
    for e in range(E):
        n_e = len(toks[e])
        ye = np.asarray(res.results[e]["y"][:, :n_e], np.float32)  # [H, n_e]
        out[toks[e]] += ws[e][:, None] * ye.T
    return out.reshape(Bb, S, Hd), res


def kernel(x, gate_w, W1, b1, W2, b2):
    out, _ = _run({"x": x, "gate_w": gate_w, "W1": W1, "b1": b1,
                   "W2": W2, "b2": b2})
    return out.astype(np.asarray(x).dtype, copy=False)


# Exposed for test.py: run with profiling, return (output, BassKernelResults)
def kernel_profiled(x, gate_w, W1, b1, W2, b2):
    return _run({"x": x, "gate_w": gate_w, "W1": W1, "b1": b1,
                 "W2": W2, "b2": b2}, trace=True)
